# revision 5
# baseline (speedup 1.0000x reference)
"""MoE top-1 routed expert FFN (8 experts) on 8 Trainium2 NeuronCores.

Strategy: expert parallelism. Core e holds expert e's weights. The host
computes the token->expert permutation (top-1 dispatch is just a gather),
ships each core its tokens transposed (tokens on the matmul free dim),
and the device runs the whole FFN in transposed token space:

    hT = gelu_tanh(w1_tile.T @ xT + b1)        (per 128-wide ff tile)
    yT = sum_ff w2_tile.T @ hT + b2            (accumulated in PSUM)

so w1 ([D, FF]) and w2 ([FF, D]) act as PE stationary operands in their
natural layouts and no on-device transpose is needed. The host scatters
each core's yT back into the full output (tokens are disjoint across
experts, so the source's all-reduce degenerates to a scatter).

Matmul operands are fp16 (fast weight load + full-rate PE, ~5e-4 rel err)
with fp32 PSUM accumulation. Weights are packed host-side into per-ff-pair
slabs so each DMA is large and in PE consumption order. A burst of dummy
matmuls at kernel start warms the PE clock (HAM) while the first DMAs are
in flight.
"""

import os

import numpy as np

import concourse.mybir as mybir
import concourse.tile as tile
from concourse import bacc, bass_utils

N_CORES = 8
D = 768
FF = 3072
KD = D // 128  # 6
KF = FF // 128  # 24
NPACK = KF // 2  # two ff-tiles of (w1 slab | w2 tile) per DMA pack

_compiled = {}


def _maybe_trace():
    """Enable NTFF tracing only when MOE_TRACE=1 and the axon profile hook
    can be installed. The graded path never sets the env var."""
    if not os.environ.get("MOE_TRACE"):
        return False
    try:
        import sys
        import types

        if "antenv.axon_hooks" not in sys.modules:
            mod = types.ModuleType("antenv.axon_hooks")
            _h = [None]
            mod.set_axon_ntff_profile_hook = lambda h: _h.__setitem__(0, h)
            mod.get_axon_ntff_profile_hook = lambda: _h[0]
            sys.modules["antenv.axon_hooks"] = mod
            from trn_agent_boot.trn_boot import _ntff_profile_via_ctypes

            mod.set_axon_ntff_profile_hook(
                _ntff_profile_via_ctypes("/opt/axon/libaxon_pjrt.so")
            )
        return True
    except Exception:
        return False


def _build(chunks):
    """Build + compile the per-core FFN kernel for token chunk sizes `chunks`."""
    C = sum(chunks)
    f32 = mybir.dt.float32
    f16 = mybir.dt.float16
    gelu = mybir.ActivationFunctionType.Gelu_apprx_tanh
    ident = mybir.ActivationFunctionType.Identity

    nc = bacc.Bacc("TRN2", target_bir_lowering=False, debug=False, num_devices=N_CORES)
    # xp[p, k*C + c] = x[token c, k*128 + p]
    xp_d = nc.dram_tensor("xp", [128, KD * C], f16, kind="ExternalInput").ap()
    # wp[i]: [w1h(2i) | w2(2i) | w1h(2i+1) | w2(2i+1)], each [128, 768] lhsT slab
    wp_d = nc.dram_tensor("wp", [NPACK, 128, 4 * D], f16, kind="ExternalInput").ap()
    # bp[:, :KF] = b1 tiles, bp[:, KF:KF+KD] = b2 tiles
    bp_d = nc.dram_tensor("bp", [128, KF + KD], f32, kind="ExternalInput").ap()
    yT_d = nc.dram_tensor("yT", [D, C], f32, kind="ExternalOutput").ap()

    with tile.TileContext(nc) as tc:
        with (
            tc.tile_pool(name="wpool", bufs=1) as wpool,
            tc.tile_pool(name="xpool", bufs=1) as xpool,
            tc.tile_pool(name="hpool", bufs=3) as hpool,
            tc.tile_pool(name="ypool", bufs=3) as ypool,
            tc.tile_pool(name="bpool", bufs=1) as bpool,
            tc.tile_pool(name="phpool", bufs=2, space="PSUM") as phpool,
            tc.tile_pool(name="pypool", bufs=1, space="PSUM") as pypool,
        ):
            # PE warmup: dummy matmuls with no DMA dependency keep the PE busy
            # through the HAM activity window while input DMAs are in flight,
            # so the real matmul stream starts at full clock.
            warm_w = bpool.tile([128, 128], f16, tag="warm")
            nc.vector.memset(warm_w[:], 0.0)
            warm_ps = phpool.tile([128, chunks[0]], f32, tag="ph", name="warm_ps")
            for _ in range(32):
                nc.tensor.matmul(
                    warm_ps[:, :128], warm_w[:], warm_w[:], start=True, stop=True
                )

            # input DMAs; xp + first weight pack issue on Scalar's HWDGE queue
            # in parallel with Sync issuing the rest
            x_sb = xpool.tile([128, KD * C], f16, tag="x")
            nc.scalar.dma_start(x_sb[:], xp_d)
            w_sb = []
            for i in range(NPACK):
                t = wpool.tile([128, 4 * D], f16, tag=f"wp{i}", name=f"wp{i}")
                (nc.scalar if i == 0 else nc.sync).dma_start(t[:], wp_d[i, :, :])
                w_sb.append(t)
            b_sb = bpool.tile([128, KF + KD], f32, tag="b")
            nc.sync.dma_start(b_sb[:], bp_d)

            c0 = 0
            for ci, Cc in enumerate(chunks):
                py = [
                    pypool.tile([128, Cc], f32, tag=f"py{d}", name=f"py{d}_{ci}")
                    for d in range(KD)
                ]
                for ff in range(KF):
                    wt = w_sb[ff // 2]
                    off = (ff % 2) * 2 * D
                    ph = phpool.tile([128, Cc], f32, tag="ph")
                    for k in range(KD):
                        nc.tensor.matmul(
                            ph[:],
                            wt[:, off + k * 128 : off + (k + 1) * 128],
                            x_sb[:, k * C + c0 : k * C + c0 + Cc],
                            start=(k == 0),
                            stop=(k == KD - 1),
                        )
                    h_sb = hpool.tile([128, Cc], f16, tag="h")
                    nc.scalar.activation(
                        h_sb[:], ph[:], gelu, bias=b_sb[:, ff : ff + 1], scale=1.0
                    )
                    for d in range(KD):
                        nc.tensor.matmul(
                            py[d][:],
                            wt[:, off + D + d * 128 : off + D + (d + 1) * 128],
                            h_sb[:],
                            start=(ff == 0),
                            stop=(ff == KF - 1),
                        )
                for d in range(KD):
                    y_sb = ypool.tile([128, Cc], f32, tag="y")
                    b2ap = b_sb[:, KF + d : KF + d + 1]
                    if d % 2 == 0:
                        nc.vector.tensor_scalar_add(y_sb[:], py[d][:], b2ap)
                    else:
                        nc.scalar.activation(y_sb[:], py[d][:], ident, bias=b2ap)
                    nc.sync.dma_start(
                        yT_d[d * 128 : (d + 1) * 128, c0 : c0 + Cc], y_sb[:]
                    )
                c0 += Cc
    nc.compile()
    return nc


def _get_compiled(chunks):
    key = tuple(chunks)
    if key not in _compiled:
        _compiled[key] = _build(list(key))
    return _compiled[key]


def kernel(inputs, dispatch_order, w1, b1, w2, b2):
    x = np.asarray(inputs, dtype=np.float32)
    B, S, Dm = x.shape
    T = B * S
    xf = x.reshape(T, Dm)
    disp = np.asarray(dispatch_order).astype(np.int64)
    w1 = np.asarray(w1, dtype=np.float32)
    b1 = np.asarray(b1, dtype=np.float32)
    w2 = np.asarray(w2, dtype=np.float32)
    b2 = np.asarray(b2, dtype=np.float32)
    E = w1.shape[0]

    counts = np.bincount(disp, minlength=E)
    cmax = max(int(counts.max()), 16)
    # token capacity per core: chunks of <=512 (PSUM bank limit for fp32
    # accumulation), balanced so the PE moving dim stays large
    n_chunks = -(-cmax // 512)
    cc = -(-(-(-cmax // n_chunks)) // 16) * 16
    chunks = [cc] * n_chunks
    C = cc * n_chunks

    order = np.argsort(disp, kind="stable")
    starts = np.concatenate([[0], np.cumsum(counts)])

    in_maps = []
    for e in range(E):
        ids = order[starts[e] : starts[e + 1]]
        xe = np.zeros((C, Dm), dtype=np.float32)
        xe[: len(ids)] = xf[ids]
        xp = xe.reshape(C, KD, 128).transpose(2, 1, 0).reshape(128, KD * C)
        # w1 in lhsT slab layout: w1h[ff][p, k*128+c] = w1[k*128+p, ff*128+c]
        w1h = (
            w1[e]
            .reshape(KD, 128, KF, 128)
            .transpose(2, 1, 0, 3)
            .reshape(KF, 128, KD * 128)
        )
        w2t = w2[e].reshape(KF, 128, D)
        wp = (
            np.concatenate([w1h, w2t], axis=2)
            .reshape(NPACK, 2, 128, 2 * D)
            .transpose(0, 2, 1, 3)
            .reshape(NPACK, 128, 4 * D)
        )
        bp = np.concatenate(
            [b1[e].reshape(KF, 128).T, b2[e].reshape(KD, 128).T], axis=1
        )
        in_maps.append(
            {
                "xp": np.ascontiguousarray(xp).astype(np.float16),
                "wp": np.ascontiguousarray(wp).astype(np.float16),
                "bp": np.ascontiguousarray(bp),
            }
        )

    nc = _get_compiled(chunks)
    res = bass_utils.run_bass_kernel_spmd(
        nc, in_maps, core_ids=list(range(N_CORES)), trace=_maybe_trace()
    )
    if res.exec_time_ns is not None:
        print(f"HW exec time: {res.exec_time_ns} ns")
        if res.instructions_and_trace is not None:
            print(f"trace: {res.instructions_and_trace[1]}")

    out = np.zeros((T, Dm), dtype=np.float32)
    for e in range(E):
        ids = order[starts[e] : starts[e + 1]]
        yT = res.results[e]["yT"]
        out[ids] = yT[:, : len(ids)].T
    return out.reshape(B, S, Dm)


# revision 6
# speedup vs baseline: 1.3028x; 1.3028x over previous
"""MoE top-1 routed expert FFN (8 experts) on 8 Trainium2 NeuronCores.

Strategy: expert parallelism. Core e holds expert e's weights. The host
computes the token->expert permutation (top-1 dispatch is just a gather),
ships each core its tokens transposed (tokens on the matmul free dim),
and the device runs the whole FFN in transposed token space:

    hT = gelu_tanh(w1_tile.T @ xT + b1)        (per 128-wide ff tile)
    yT = sum_ff w2_tile.T @ hT + b2            (accumulated in PSUM)

so w1 ([D, FF]) and w2 ([FF, D]) act as PE stationary operands in their
natural layouts and no on-device transpose is needed. The host scatters
each core's yT back into the full output (tokens are disjoint across
experts, so the source's all-reduce degenerates to a scatter).

Matmul operands are fp16 (fast weight load + full-rate PE, ~5e-4 rel err)
with fp32 PSUM accumulation. Weights are packed host-side into per-ff-pair
slabs so each DMA is large and in PE consumption order. A burst of dummy
matmuls at kernel start warms the PE clock (HAM) while the first DMAs are
in flight.
"""

import os

import numpy as np

import concourse.mybir as mybir
import concourse.tile as tile
from concourse import bacc, bass_utils

N_CORES = 8
D = 768
FF = 3072
KD = D // 128  # 6
KF = FF // 128  # 24
NPACK = KF // 2  # two ff-tiles of (w1 slab | w2 tile) per DMA pack

_compiled = {}


def _maybe_trace():
    """Enable NTFF tracing only when MOE_TRACE=1 and the axon profile hook
    can be installed. The graded path never sets the env var."""
    if not os.environ.get("MOE_TRACE"):
        return False
    try:
        import sys
        import types

        if "antenv.axon_hooks" not in sys.modules:
            mod = types.ModuleType("antenv.axon_hooks")
            _h = [None]
            mod.set_axon_ntff_profile_hook = lambda h: _h.__setitem__(0, h)
            mod.get_axon_ntff_profile_hook = lambda: _h[0]
            sys.modules["antenv.axon_hooks"] = mod
            from trn_agent_boot.trn_boot import _ntff_profile_via_ctypes

            mod.set_axon_ntff_profile_hook(
                _ntff_profile_via_ctypes("/opt/axon/libaxon_pjrt.so")
            )
        return True
    except Exception:
        return False


def _build(chunks):
    """Build + compile the per-core FFN kernel for token chunk sizes `chunks`."""
    C = sum(chunks)
    f32 = mybir.dt.float32
    f16 = mybir.dt.float16
    gelu = mybir.ActivationFunctionType.Gelu_apprx_tanh
    ident = mybir.ActivationFunctionType.Identity

    nc = bacc.Bacc("TRN2", target_bir_lowering=False, debug=False, num_devices=N_CORES)
    # xp[p, k*C + c] = x[token c, k*128 + p]
    xp_d = nc.dram_tensor("xp", [128, KD * C], f16, kind="ExternalInput").ap()
    # wp[i]: [w1h(2i) | w2(2i) | w1h(2i+1) | w2(2i+1)], each [128, 768] lhsT slab
    wp_d = nc.dram_tensor("wp", [NPACK, 128, 4 * D], f16, kind="ExternalInput").ap()
    # bp[:, :KF] = b1 tiles, bp[:, KF:KF+KD] = b2 tiles
    bp_d = nc.dram_tensor("bp", [128, KF + KD], f32, kind="ExternalInput").ap()
    yT_d = nc.dram_tensor("yT", [D, C], f32, kind="ExternalOutput").ap()

    with tile.TileContext(nc) as tc:
        with (
            tc.tile_pool(name="wpool", bufs=1) as wpool,
            tc.tile_pool(name="xpool", bufs=1) as xpool,
            tc.tile_pool(name="hpool", bufs=3) as hpool,
            tc.tile_pool(name="ypool", bufs=3) as ypool,
            tc.tile_pool(name="bpool", bufs=1) as bpool,
            tc.tile_pool(name="phpool", bufs=2, space="PSUM") as phpool,
            tc.tile_pool(name="pypool", bufs=1, space="PSUM") as pypool,
        ):
            # PE warmup: dummy matmuls with no DMA dependency keep the PE busy
            # through the HAM activity window while input DMAs are in flight,
            # so the real matmul stream starts at full clock.
            warm_w = bpool.tile([128, 128], f16, tag="warm")
            nc.vector.memset(warm_w[:], 0.0)
            warm_ps = phpool.tile([128, chunks[0]], f32, tag="ph", name="warm_ps")
            for _ in range(32):
                nc.tensor.matmul(
                    warm_ps[:, :128], warm_w[:], warm_w[:], start=True, stop=True
                )
            # preload the gelu PWL table off the critical path
            warm_h = bpool.tile([128, 16], f16, tag="warmh")
            nc.scalar.activation(warm_h[:], warm_w[:, :16], gelu, bias=0.0, scale=1.0)

            # input DMAs, queued in first-use order: xp on Scalar's HWDGE ring
            # concurrently with wp0 + bias + remaining packs on Sync's ring
            x_sb = xpool.tile([128, KD * C], f16, tag="x")
            nc.scalar.dma_start(x_sb[:], xp_d)
            w_sb = []
            b_sb = bpool.tile([128, KF + KD], f32, tag="b")
            for i in range(NPACK):
                t = wpool.tile([128, 4 * D], f16, tag=f"wp{i}", name=f"wp{i}")
                nc.sync.dma_start(t[:], wp_d[i, :, :])
                w_sb.append(t)
                if i == 0:
                    nc.sync.dma_start(b_sb[:], bp_d)

            c0 = 0
            for ci, Cc in enumerate(chunks):
                py = [
                    pypool.tile([128, Cc], f32, tag=f"py{d}", name=f"py{d}_{ci}")
                    for d in range(KD)
                ]
                for ff in range(KF):
                    wt = w_sb[ff // 2]
                    off = (ff % 2) * 2 * D
                    ph = phpool.tile([128, Cc], f32, tag="ph")
                    for k in range(KD):
                        nc.tensor.matmul(
                            ph[:],
                            wt[:, off + k * 128 : off + (k + 1) * 128],
                            x_sb[:, k * C + c0 : k * C + c0 + Cc],
                            start=(k == 0),
                            stop=(k == KD - 1),
                        )
                    h_sb = hpool.tile([128, Cc], f16, tag="h")
                    nc.scalar.activation(
                        h_sb[:], ph[:], gelu, bias=b_sb[:, ff : ff + 1], scale=1.0
                    )
                    for d in range(KD):
                        nc.tensor.matmul(
                            py[d][:],
                            wt[:, off + D + d * 128 : off + D + (d + 1) * 128],
                            h_sb[:],
                            start=(ff == 0),
                            stop=(ff == KF - 1),
                        )
                for d in range(KD):
                    y_sb = ypool.tile([128, Cc], f32, tag="y")
                    b2ap = b_sb[:, KF + d : KF + d + 1]
                    if d % 2 == 0:
                        nc.vector.tensor_scalar_add(y_sb[:], py[d][:], b2ap)
                    else:
                        nc.scalar.activation(y_sb[:], py[d][:], ident, bias=b2ap)
                    nc.sync.dma_start(
                        yT_d[d * 128 : (d + 1) * 128, c0 : c0 + Cc], y_sb[:]
                    )
                c0 += Cc
    nc.compile()
    return nc


def _get_compiled(chunks):
    key = tuple(chunks)
    if key not in _compiled:
        _compiled[key] = _build(list(key))
    return _compiled[key]


def kernel(inputs, dispatch_order, w1, b1, w2, b2):
    x = np.asarray(inputs, dtype=np.float32)
    B, S, Dm = x.shape
    T = B * S
    xf = x.reshape(T, Dm)
    disp = np.asarray(dispatch_order).astype(np.int64)
    w1 = np.asarray(w1, dtype=np.float32)
    b1 = np.asarray(b1, dtype=np.float32)
    w2 = np.asarray(w2, dtype=np.float32)
    b2 = np.asarray(b2, dtype=np.float32)
    E = w1.shape[0]

    counts = np.bincount(disp, minlength=E)
    cmax = max(int(counts.max()), 16)
    # token capacity per core: chunks of <=512 (PSUM bank limit for fp32
    # accumulation), balanced so the PE moving dim stays large
    n_chunks = -(-cmax // 512)
    cc = -(-(-(-cmax // n_chunks)) // 16) * 16
    chunks = [cc] * n_chunks
    C = cc * n_chunks

    order = np.argsort(disp, kind="stable")
    starts = np.concatenate([[0], np.cumsum(counts)])

    in_maps = []
    for e in range(E):
        ids = order[starts[e] : starts[e + 1]]
        xe = np.zeros((C, Dm), dtype=np.float32)
        xe[: len(ids)] = xf[ids]
        xp = xe.reshape(C, KD, 128).transpose(2, 1, 0).reshape(128, KD * C)
        # w1 in lhsT slab layout: w1h[ff][p, k*128+c] = w1[k*128+p, ff*128+c]
        w1h = (
            w1[e]
            .reshape(KD, 128, KF, 128)
            .transpose(2, 1, 0, 3)
            .reshape(KF, 128, KD * 128)
        )
        w2t = w2[e].reshape(KF, 128, D)
        wp = (
            np.concatenate([w1h, w2t], axis=2)
            .reshape(NPACK, 2, 128, 2 * D)
            .transpose(0, 2, 1, 3)
            .reshape(NPACK, 128, 4 * D)
        )
        bp = np.concatenate(
            [b1[e].reshape(KF, 128).T, b2[e].reshape(KD, 128).T], axis=1
        )
        in_maps.append(
            {
                "xp": np.ascontiguousarray(xp).astype(np.float16),
                "wp": np.ascontiguousarray(wp).astype(np.float16),
                "bp": np.ascontiguousarray(bp),
            }
        )

    nc = _get_compiled(chunks)
    res = bass_utils.run_bass_kernel_spmd(
        nc, in_maps, core_ids=list(range(N_CORES)), trace=_maybe_trace()
    )
    if res.exec_time_ns is not None:
        print(f"HW exec time: {res.exec_time_ns} ns")
        if res.instructions_and_trace is not None:
            print(f"trace: {res.instructions_and_trace[1]}")

    out = np.zeros((T, Dm), dtype=np.float32)
    for e in range(E):
        ids = order[starts[e] : starts[e + 1]]
        yT = res.results[e]["yT"]
        out[ids] = yT[:, : len(ids)].T
    return out.reshape(B, S, Dm)


# revision 7
# speedup vs baseline: 1.3205x; 1.0136x over previous
"""MoE top-1 routed expert FFN (8 experts) on 8 Trainium2 NeuronCores.

Strategy: expert parallelism. Core e holds expert e's weights. The host
computes the token->expert permutation (top-1 dispatch is just a gather),
ships each core its tokens transposed (tokens on the matmul free dim),
and the device runs the whole FFN in transposed token space:

    hT = gelu_tanh(w1_tile.T @ xT + b1)        (per 128-wide ff tile)
    yT = sum_ff w2_tile.T @ hT + b2            (accumulated in PSUM)

so w1 ([D, FF]) and w2 ([FF, D]) act as PE stationary operands in their
natural layouts and no on-device transpose is needed. The host scatters
each core's yT back into the full output (tokens are disjoint across
experts, so the source's all-reduce degenerates to a scatter).

Matmul operands are fp16 (fast weight load + full-rate PE, ~5e-4 rel err)
with fp32 PSUM accumulation. Weights are packed host-side into per-ff-pair
slabs so each DMA is large and in PE consumption order. A burst of dummy
matmuls at kernel start warms the PE clock (HAM) while the first DMAs are
in flight.
"""

import os

import numpy as np

import concourse.mybir as mybir
import concourse.tile as tile
from concourse import bacc, bass_utils

N_CORES = 8
D = 768
FF = 3072
KD = D // 128  # 6
KF = FF // 128  # 24
NPACK = KF  # one ff-tile of (w1 slab | w2 tile) per DMA pack

_compiled = {}


def _maybe_trace():
    """Enable NTFF tracing only when MOE_TRACE=1 and the axon profile hook
    can be installed. The graded path never sets the env var."""
    if not os.environ.get("MOE_TRACE"):
        return False
    try:
        import sys
        import types

        if "antenv.axon_hooks" not in sys.modules:
            mod = types.ModuleType("antenv.axon_hooks")
            _h = [None]
            mod.set_axon_ntff_profile_hook = lambda h: _h.__setitem__(0, h)
            mod.get_axon_ntff_profile_hook = lambda: _h[0]
            sys.modules["antenv.axon_hooks"] = mod
            from trn_agent_boot.trn_boot import _ntff_profile_via_ctypes

            mod.set_axon_ntff_profile_hook(
                _ntff_profile_via_ctypes("/opt/axon/libaxon_pjrt.so")
            )
        return True
    except Exception:
        return False


def _build(chunks):
    """Build + compile the per-core FFN kernel for token chunk sizes `chunks`."""
    C = sum(chunks)
    f32 = mybir.dt.float32
    f16 = mybir.dt.float16
    gelu = mybir.ActivationFunctionType.Gelu_apprx_tanh
    ident = mybir.ActivationFunctionType.Identity

    nc = bacc.Bacc("TRN2", target_bir_lowering=False, debug=False, num_devices=N_CORES)
    # xp[p, k*C + c] = x[token c, k*128 + p]
    xp_d = nc.dram_tensor("xp", [128, KD * C], f16, kind="ExternalInput").ap()
    # wp[ff]: [w1h(ff) | w2(ff)], each half a [128, 768] lhsT slab
    wp_d = nc.dram_tensor("wp", [NPACK, 128, 2 * D], f16, kind="ExternalInput").ap()
    # bp[:, :KF] = b1 tiles, bp[:, KF:KF+KD] = b2 tiles
    bp_d = nc.dram_tensor("bp", [128, KF + KD], f32, kind="ExternalInput").ap()
    yT_d = nc.dram_tensor("yT", [D, C], f32, kind="ExternalOutput").ap()

    with tile.TileContext(nc) as tc:
        with (
            tc.tile_pool(name="wpool", bufs=1) as wpool,
            tc.tile_pool(name="xpool", bufs=1) as xpool,
            tc.tile_pool(name="hpool", bufs=3) as hpool,
            tc.tile_pool(name="ypool", bufs=3) as ypool,
            tc.tile_pool(name="bpool", bufs=1) as bpool,
            tc.tile_pool(name="phpool", bufs=2, space="PSUM") as phpool,
            tc.tile_pool(name="pypool", bufs=1, space="PSUM") as pypool,
        ):
            # PE warmup: dummy matmuls with no DMA dependency keep the PE busy
            # through the HAM activity window while input DMAs are in flight,
            # so the real matmul stream starts at full clock.
            warm_w = bpool.tile([128, 128], f16, tag="warm")
            nc.vector.memset(warm_w[:], 0.0)
            warm_ps = phpool.tile([128, chunks[0]], f32, tag="ph", name="warm_ps")
            for _ in range(16):
                nc.tensor.matmul(
                    warm_ps[:, :128], warm_w[:], warm_w[:], start=True, stop=True
                )

            # input DMAs, queued in first-use order and split across the two
            # HWDGE rings (Scalar + Sync) so the first tiles land early
            x_sb = xpool.tile([128, KD * C], f16, tag="x")
            half = 3 * C
            nc.scalar.dma_start(x_sb[:, :half], xp_d[:, :half])
            # preload the gelu PWL table off the critical path (after the xp
            # issue: the lazy ACT_TABLE_LOAD costs ~1.3us on Scalar)
            warm_h = bpool.tile([128, 16], f16, tag="warmh")
            nc.scalar.activation(warm_h[:], warm_w[:, :16], gelu, bias=0.0, scale=1.0)
            w_sb = []
            b_sb = bpool.tile([128, KF + KD], f32, tag="b")
            for i in range(NPACK):
                t = wpool.tile([128, 2 * D], f16, tag=f"wp{i}", name=f"wp{i}")
                nc.sync.dma_start(t[:], wp_d[i, :, :])
                w_sb.append(t)
                if i == 0:
                    nc.sync.dma_start(x_sb[:, half:], xp_d[:, half:])
                    nc.sync.dma_start(b_sb[:], bp_d)

            c0 = 0
            for ci, Cc in enumerate(chunks):
                py = [
                    pypool.tile([128, Cc], f32, tag=f"py{d}", name=f"py{d}_{ci}")
                    for d in range(KD)
                ]
                for ff in range(KF):
                    wt = w_sb[ff]
                    off = 0
                    ph = phpool.tile([128, Cc], f32, tag="ph")
                    for k in range(KD):
                        nc.tensor.matmul(
                            ph[:],
                            wt[:, off + k * 128 : off + (k + 1) * 128],
                            x_sb[:, k * C + c0 : k * C + c0 + Cc],
                            start=(k == 0),
                            stop=(k == KD - 1),
                        )
                    h_sb = hpool.tile([128, Cc], f16, tag="h")
                    nc.scalar.activation(
                        h_sb[:], ph[:], gelu, bias=b_sb[:, ff : ff + 1], scale=1.0
                    )
                    for d in range(KD):
                        nc.tensor.matmul(
                            py[d][:],
                            wt[:, off + D + d * 128 : off + D + (d + 1) * 128],
                            h_sb[:],
                            start=(ff == 0),
                            stop=(ff == KF - 1),
                        )
                for d in range(KD):
                    y_sb = ypool.tile([128, Cc], f32, tag="y")
                    b2ap = b_sb[:, KF + d : KF + d + 1]
                    if d % 2 == 0:
                        nc.vector.tensor_scalar_add(y_sb[:], py[d][:], b2ap)
                    else:
                        nc.scalar.activation(y_sb[:], py[d][:], ident, bias=b2ap)
                    (nc.sync if d % 2 == 0 else nc.scalar).dma_start(
                        yT_d[d * 128 : (d + 1) * 128, c0 : c0 + Cc], y_sb[:]
                    )
                c0 += Cc
    nc.compile()
    return nc


def _get_compiled(chunks):
    key = tuple(chunks)
    if key not in _compiled:
        _compiled[key] = _build(list(key))
    return _compiled[key]


def kernel(inputs, dispatch_order, w1, b1, w2, b2):
    x = np.asarray(inputs, dtype=np.float32)
    B, S, Dm = x.shape
    T = B * S
    xf = x.reshape(T, Dm)
    disp = np.asarray(dispatch_order).astype(np.int64)
    w1 = np.asarray(w1, dtype=np.float32)
    b1 = np.asarray(b1, dtype=np.float32)
    w2 = np.asarray(w2, dtype=np.float32)
    b2 = np.asarray(b2, dtype=np.float32)
    E = w1.shape[0]

    counts = np.bincount(disp, minlength=E)
    cmax = max(int(counts.max()), 16)
    # token capacity per core: chunks of <=512 (PSUM bank limit for fp32
    # accumulation), balanced so the PE moving dim stays large
    n_chunks = -(-cmax // 512)
    cc = -(-(-(-cmax // n_chunks)) // 16) * 16
    chunks = [cc] * n_chunks
    C = cc * n_chunks

    order = np.argsort(disp, kind="stable")
    starts = np.concatenate([[0], np.cumsum(counts)])

    in_maps = []
    for e in range(E):
        ids = order[starts[e] : starts[e + 1]]
        xe = np.zeros((C, Dm), dtype=np.float32)
        xe[: len(ids)] = xf[ids]
        xp = xe.reshape(C, KD, 128).transpose(2, 1, 0).reshape(128, KD * C)
        # w1 in lhsT slab layout: w1h[ff][p, k*128+c] = w1[k*128+p, ff*128+c]
        w1h = (
            w1[e]
            .reshape(KD, 128, KF, 128)
            .transpose(2, 1, 0, 3)
            .reshape(KF, 128, KD * 128)
        )
        w2t = w2[e].reshape(KF, 128, D)
        wp = np.concatenate([w1h, w2t], axis=2)
        bp = np.concatenate(
            [b1[e].reshape(KF, 128).T, b2[e].reshape(KD, 128).T], axis=1
        )
        in_maps.append(
            {
                "xp": np.ascontiguousarray(xp).astype(np.float16),
                "wp": np.ascontiguousarray(wp).astype(np.float16),
                "bp": np.ascontiguousarray(bp),
            }
        )

    nc = _get_compiled(chunks)
    res = bass_utils.run_bass_kernel_spmd(
        nc, in_maps, core_ids=list(range(N_CORES)), trace=_maybe_trace()
    )
    if res.exec_time_ns is not None:
        print(f"HW exec time: {res.exec_time_ns} ns")
        if res.instructions_and_trace is not None:
            print(f"trace: {res.instructions_and_trace[1]}")

    out = np.zeros((T, Dm), dtype=np.float32)
    for e in range(E):
        ids = order[starts[e] : starts[e + 1]]
        yT = res.results[e]["yT"]
        out[ids] = yT[:, : len(ids)].T
    return out.reshape(B, S, Dm)
